# revision 9
# baseline (speedup 1.0000x reference)
"""Trainium2 Bass kernel for nn_NodeAggregator (gnn message passing / diffpool-style).

Reference math (per batch element b, forward pass only):
    h      = relu(x @ W1 + b1)                      [N, K]
    logits = h @ W2 + b2 + (-1e9)*(1-mask)[:,None]  [N, K]
    S      = softmax(logits, axis=-1)               [N, K]
    pfeat  = S.T @ x                                [K, F]
    pooled = S.T @ adj @ S                          [K, K]
    (threshold/topk/scatter + straight-through estimator is an exact
     no-op in the forward pass: a_sp + (pooled - a_sp) == pooled)
    d      = 1/sqrt(pooled.sum(-1) + 1e-9)
    padj   = pooled * d[:,None] * d[None,:]
    pmask  = ones

Sharding: data-parallel over batch B=8 across the 8 NeuronCores (one batch
element per core); weight matrices replicated. No collectives.

Layout trick: everything stays in natural (row-major) orientation by
re-associating pooled = (adj.T @ S).T @ S, so the adjacency tiles serve
directly as matmul stationary operands and no on-device transpose is ever
needed.  x is additionally passed pre-transposed from the host (xT) for the
h-stage, whose contraction runs over F.

dtypes: the h/logits/softmax/pfeat path runs in fp32 (float32r matmul mode:
full fp32 precision at 1 cycle/row for free-dim >= 256).  The big
adj-contraction (85% of FLOPs and HBM bytes) runs with bf16 inputs and fp32
PSUM accumulation; because adj >= 0 and S >= 0 those sums average out the
rounding noise (~5e-4 relative on padj).
"""

import os
from contextlib import ExitStack

import ml_dtypes
import numpy as np

import concourse.bass as bass
import concourse.tile as tile
from concourse import bacc, mybir
from concourse.masks import make_identity
import concourse.bass_utils as _bu
from concourse.bass_utils import run_bass_kernel_spmd

# The walrus driver is invoked with --enable-ldw-opt=false hardcoded, which
# leaves every LDWEIGHTS serialized with its matmul (~35% of PE time here).
# NK_LDW_OPT=1 flips the flag (experiment; verify correctness!).
if os.environ.get("NK_LDW_OPT", "0") == "1" and not getattr(_bu, "_nk_patched", False):
    _orig_run_command = _bu.run_command

    def _nk_run_command(argv, **kwargs):
        argv = [
            "--enable-ldw-opt=true" if a == "--enable-ldw-opt=false" else a
            for a in argv
        ]
        return _orig_run_command(argv, **kwargs)

    _bu.run_command = _nk_run_command
    _bu._nk_patched = True

B, N, F, K = 8, 2048, 512, 256
P = 128
NT = N // P   # 16 n-tiles
FT = F // P   # 4 f-tiles
KH = K // P   # 2 k-halves
NCH = 4       # n-chunks for the h-stage (512 wide)
CH = N // NCH
TT_PHASES = int(os.environ.get("NK_TT_PHASES", "4"))
MTP = NT // TT_PHASES  # m-tiles per TT phase
MW = N // TT_PHASES    # m columns per TT phase

F32 = mybir.dt.float32
F32R = mybir.dt.float32r
BF16 = mybir.dt.bfloat16
X = mybir.AxisListType.X
AF = mybir.ActivationFunctionType

# Accuracy/perf knobs. NK_ADJ_F32=1 switches the adj contraction to fp32
# (float32r) end to end: ~1.4x slower (HBM bound) but fully fp32-exact.
ADJ_F32 = os.environ.get("NK_ADJ_F32", "0") == "1"
DT_A = F32 if ADJ_F32 else BF16
NP_A = np.float32 if ADJ_F32 else ml_dtypes.bfloat16


def _mm_dt(ap):
    """Bitcast fp32 matmul operands to float32r (fast fp32 PE mode)."""
    return ap.bitcast(F32R) if ap.dtype == F32 else ap


def _build_kernel(ctx: ExitStack, tc: tile.TileContext, io: dict):
    nc = tc.nc

    consts = ctx.enter_context(tc.tile_pool(name="consts", bufs=1))
    big = ctx.enter_context(tc.tile_pool(name="big", bufs=1))
    apool = ctx.enter_context(tc.tile_pool(name="apool", bufs=6))
    sm = ctx.enter_context(tc.tile_pool(name="sm", bufs=4))
    work = ctx.enter_context(tc.tile_pool(name="work", bufs=3))
    evp = ctx.enter_context(tc.tile_pool(name="evp", bufs=3))
    psum = ctx.enter_context(tc.tile_pool(name="psum", bufs=2, space="PSUM"))
    psum_tt = ctx.enter_context(tc.tile_pool(name="psum_tt", bufs=1, space="PSUM"))

    # ---- constants / weights ----
    w1_sb = consts.tile([P, FT, K], F32R)
    nc.sync.dma_start(w1_sb, io["w1"].rearrange("(ft p) k -> p ft k", p=P))
    w2_sb = consts.tile([P, KH, K], F32R)
    nc.sync.dma_start(w2_sb, io["w2"].rearrange("(h p) k -> p h k", p=P))
    b1_sb = consts.tile([P, KH], F32)
    nc.sync.dma_start(b1_sb, io["b1"].rearrange("(h p) -> p h", p=P))
    b2b_sb = consts.tile([P, K], F32)
    nc.sync.dma_start(b2b_sb, io["b2"].partition_broadcast(P))
    mb_sb = consts.tile([P, NT], F32)
    nc.sync.dma_start(mb_sb, io["mb"].rearrange("(t p) -> p t", p=P))
    eps_sb = consts.tile([1, 1], F32)
    nc.vector.memset(eps_sb, 1e-9)
    ones_a = consts.tile([P, 1], DT_A)
    nc.vector.memset(ones_a, 1.0)
    d_sb = consts.tile([1, K], F32R)

    # ---- big resident tensors ----
    xT_sb = big.tile([P, FT, N], F32R)
    xT_r = io["xT"].rearrange("(ft p) n -> p ft n", p=P)
    for c in range(NCH):
        nc.sync.dma_start(
            xT_sb[:, :, c * CH:(c + 1) * CH], xT_r[:, :, c * CH:(c + 1) * CH]
        )
    x_sb = big.tile([P, NT, F], F32R)
    x_r = io["x"].rearrange("(t p) f -> p t f", p=P)
    for c in range(2):
        nc.sync.dma_start(
            x_sb[:, c * 8:(c + 1) * 8, :], x_r[:, c * 8:(c + 1) * 8, :]
        )
    hT_sb = big.tile([P, KH, N], F32R)
    S_sb = big.tile([P, NT, K], F32R)
    Sa_sb = big.tile([P, NT, K], DT_A)   # S cast for the adj contraction
    T_sb = big.tile([P, KH, N], DT_A)    # S.T @ adj, k on partitions
    TT_sb = big.tile([P, NT, K], DT_A)   # its transpose, m on partitions
    pooled_sb = big.tile([P, KH, K], F32)
    ident_sb = consts.tile([P, P], DT_A)
    make_identity(nc, ident_sb)

    # ---- phase 1: hT[k, n] = relu(W1.T @ xT + b1) ----
    for kh in range(KH):
        for nch in range(NCH):
            ps = psum.tile([P, CH], F32, name="acc")
            for ft in range(FT):
                nc.tensor.matmul(
                    ps,
                    _mm_dt(w1_sb[:, ft, kh * P:(kh + 1) * P]),
                    _mm_dt(xT_sb[:, ft, nch * CH:(nch + 1) * CH]),
                    start=(ft == 0),
                    stop=(ft == FT - 1),
                )
            nc.scalar.activation(
                hT_sb[:, kh, nch * CH:(nch + 1) * CH], ps, AF.Relu,
                bias=b1_sb[:, kh:kh + 1], scale=1.0,
            )

    # ---- phase 2: logits = hT.T @ W2 + b2; S = softmax(logits + maskbias) ----
    for nt in range(NT):
        lp = psum.tile([P, K], F32, name="lg")
        for kh in range(KH):
            nc.tensor.matmul(
                lp,
                _mm_dt(hT_sb[:, kh, nt * P:(nt + 1) * P]),
                _mm_dt(w2_sb[:, kh, :]),
                start=(kh == 0),
                stop=(kh == KH - 1),
            )
        lg = work.tile([P, K], F32, name="lg_sb")
        nc.vector.tensor_add(lg, lp, b2b_sb)
        mx = sm.tile([P, 1], F32, name="mx")
        nc.vector.reduce_max(mx, lg, axis=X)
        eb = sm.tile([P, 1], F32, name="eb")
        nc.vector.tensor_sub(eb, mb_sb[:, nt:nt + 1], mx)  # maskbias - max
        ex = work.tile([P, K], F32, name="ex")
        ssum = sm.tile([P, 1], F32, name="ssum")
        nc.scalar.activation(ex, lg, AF.Exp, bias=eb, scale=1.0, accum_out=ssum)
        rs = sm.tile([P, 1], F32, name="rs")
        nc.vector.reciprocal(rs, ssum)
        nc.vector.tensor_scalar_mul(S_sb[:, nt, :], ex, rs)
        nc.vector.tensor_copy(Sa_sb[:, nt, :], S_sb[:, nt, :])

    # ---- phase 3: pfeat = S.T @ x (fp32 exact) ----
    for kh in range(KH):
        ps = psum.tile([P, F], F32, name="acc")
        for nt in range(NT):
            nc.tensor.matmul(
                ps,
                _mm_dt(S_sb[:, nt, kh * P:(kh + 1) * P]),
                _mm_dt(x_sb[:, nt, :]),
                start=(nt == 0),
                stop=(nt == NT - 1),
            )
        pe = evp.tile([P, F], F32, name="pf_ev")
        nc.vector.tensor_copy(pe, ps)
        nc.sync.dma_start(io["pfeat"][kh * P:(kh + 1) * P, :], pe)

    # ---- phase 4: T[k, m] = S.T @ adj with S stationary (few, reused
    # LDWEIGHTS) and adj as the wide moving operand; adj is streamed in
    # 1024-column half-slabs.  T is evicted as bf16 and PE-transposed
    # tile-by-tile into TT[m, k] for the pooled contraction. ----
    MH = 2
    MHW = N // MH  # 1024 adj columns per half
    MC = MHW // 512  # 512-wide matmul chunks per half
    for mh in range(MH):
        tacc = {
            (kh, mc): psum_tt.tile([P, 512], F32, name=f"T{kh}{mc}")
            for kh in range(KH)
            for mc in range(MC)
        }
        for nt in range(NT):
            at = apool.tile([P, MHW], DT_A, name="aslab")
            nc.sync.dma_start(
                at, io["adj"][nt * P:(nt + 1) * P, mh * MHW:(mh + 1) * MHW]
            )
            for kh in range(KH):
                for mc in range(MC):
                    nc.tensor.matmul(
                        tacc[(kh, mc)],
                        _mm_dt(Sa_sb[:, nt, kh * P:(kh + 1) * P]),
                        _mm_dt(at[:, mc * 512:(mc + 1) * 512]),
                        start=(nt == 0),
                        stop=(nt == NT - 1),
                    )
        for kh in range(KH):
            for mc in range(MC):
                nc.vector.tensor_copy(
                    T_sb[:, kh, mh * MHW + mc * 512:mh * MHW + (mc + 1) * 512],
                    tacc[(kh, mc)],
                )
    # transpose T -> TT (PE transpose, bf16, one [128,128] block at a time)
    for mt in range(NT):
        for kh in range(KH):
            tp = psum.tile([P, K], DT_A, name="lg")
            nc.tensor.transpose(
                tp[:, :P], T_sb[:, kh, mt * P:(mt + 1) * P], ident_sb
            )
            nc.vector.tensor_copy(TT_sb[:, mt, kh * P:(kh + 1) * P], tp[:, :P])

    # ---- phase 5: pooled = TT.T @ S ----
    pooled_ps = []
    for kh in range(KH):
        pp = psum.tile([P, K], F32, name="lg")
        for mt in range(NT):
            nc.tensor.matmul(
                pp,
                _mm_dt(TT_sb[:, mt, kh * P:(kh + 1) * P]),
                _mm_dt(Sa_sb[:, mt, :]),
                start=(mt == 0),
                stop=(mt == NT - 1),
            )
        pooled_ps.append(pp)
    for kh in range(KH):
        nc.vector.tensor_copy(pooled_sb[:, kh, :], pooled_ps[kh])

    # ---- phase 6: symmetric degree renorm ----
    # row_sum[k] = sum_l pooled[k,l] = sum_m TT[m,k] (S rows sum to 1),
    # computed directly as a row vector so no transpose is needed.
    rst = psum.tile([P, K], F32, name="lg")
    rsp = rst[:1, :]
    for mt in range(NT):
        nc.tensor.matmul(
            rsp, ones_a, _mm_dt(TT_sb[:, mt, :]),
            start=(mt == 0), stop=(mt == NT - 1),
        )
    d_f32 = work.tile([1, K], F32, name="d_f32")
    nc.scalar.activation(d_f32, rsp, AF.Sqrt, bias=eps_sb, scale=1.0)
    nc.vector.reciprocal(d_f32, d_f32)  # d = 1/sqrt(row_sum + eps), [1, K]
    nc.vector.tensor_copy(d_sb, d_f32)

    for kh in range(KH):
        dd = psum.tile([P, K], F32, name="lg")
        # dd[i, j] = d[kh*P+i] * d[j]  (outer product via K=1 matmul)
        nc.tensor.matmul(
            dd, d_sb[:1, kh * P:(kh + 1) * P], d_sb[:1, :], start=True, stop=True
        )
        pa = evp.tile([P, K], F32, name="pa_ev")
        nc.vector.tensor_mul(pa, pooled_sb[:, kh, :], dd)
        nc.sync.dma_start(io["padj"][kh * P:(kh + 1) * P, :], pa)


_CACHE = {}


def _get_nc():
    key = "nc"
    if key in _CACHE:
        return _CACHE[key]
    nc = bacc.Bacc(
        "TRN2", target_bir_lowering=False, debug=False, enable_asserts=True
    )
    io = {
        "xT": nc.dram_tensor("xT", [F, N], F32R, kind="ExternalInput").ap(),
        "x": nc.dram_tensor("x", [N, F], F32R, kind="ExternalInput").ap(),
        "adj": nc.dram_tensor("adj", [N, N], DT_A, kind="ExternalInput").ap(),
        "w1": nc.dram_tensor("w1", [F, K], F32R, kind="ExternalInput").ap(),
        "w2": nc.dram_tensor("w2", [K, K], F32R, kind="ExternalInput").ap(),
        "b1": nc.dram_tensor("b1", [K], F32, kind="ExternalInput").ap(),
        "b2": nc.dram_tensor("b2", [K], F32, kind="ExternalInput").ap(),
        "mb": nc.dram_tensor("mb", [N], F32, kind="ExternalInput").ap(),
        "pfeat": nc.dram_tensor("pfeat", [K, F], F32, kind="ExternalOutput").ap(),
        "padj": nc.dram_tensor("padj", [K, K], F32, kind="ExternalOutput").ap(),
    }
    with tile.TileContext(nc) as tc, ExitStack() as ctx:
        _build_kernel(ctx, tc, io)
    nc.compile()
    _CACHE[key] = nc
    return nc


def make_in_maps(x, adj, mask, W1, b1, W2, b2):
    """Build the per-core input maps from the full (unsharded) inputs."""
    x = np.asarray(x, np.float32)
    adj = np.asarray(adj, np.float32)
    mask = np.asarray(mask, np.float32)
    w1 = np.ascontiguousarray(np.asarray(W1, np.float32))
    w2 = np.ascontiguousarray(np.asarray(W2, np.float32))
    b1v = np.ascontiguousarray(np.asarray(b1, np.float32))
    b2v = np.ascontiguousarray(np.asarray(b2, np.float32))
    in_maps = []
    for b in range(B):
        xb = np.ascontiguousarray(x[b])
        in_maps.append({
            "xT": np.ascontiguousarray(xb.T),
            "x": xb,
            "adj": np.ascontiguousarray(adj[b].astype(NP_A)),
            "w1": w1,
            "w2": w2,
            "b1": b1v,
            "b2": b2v,
            "mb": np.ascontiguousarray((-1e9 * (1.0 - mask[b])).astype(np.float32)),
        })
    return in_maps


def run(x, adj, mask, W1, b1, W2, b2, trace=False):
    nc = _get_nc()
    in_maps = make_in_maps(x, adj, mask, W1, b1, W2, b2)
    res = run_bass_kernel_spmd(nc, in_maps, core_ids=list(range(B)), trace=trace)
    pfeat = np.stack([res.results[b]["pfeat"] for b in range(B)]).astype(np.float32)
    padj = np.stack([res.results[b]["padj"] for b in range(B)]).astype(np.float32)
    pmask = np.ones((B, K), np.float32)
    return (pfeat, padj, pmask), res


def kernel(x, adj, mask, W1, b1, W2, b2):
    out, _ = run(x, adj, mask, W1, b1, W2, b2, trace=False)
    return out


# revision 10
# speedup vs baseline: 1.0977x; 1.0977x over previous
"""Trainium2 Bass kernel for nn_NodeAggregator (gnn message passing / diffpool-style).

Reference math (per batch element b, forward pass only):
    h      = relu(x @ W1 + b1)                      [N, K]
    logits = h @ W2 + b2 + (-1e9)*(1-mask)[:,None]  [N, K]
    S      = softmax(logits, axis=-1)               [N, K]
    pfeat  = S.T @ x                                [K, F]
    pooled = S.T @ adj @ S                          [K, K]
    (threshold/topk/scatter + straight-through estimator is an exact
     no-op in the forward pass: a_sp + (pooled - a_sp) == pooled)
    d      = 1/sqrt(pooled.sum(-1) + 1e-9)
    padj   = pooled * d[:,None] * d[None,:]
    pmask  = ones

Sharding: data-parallel over batch B=8 across the 8 NeuronCores (one batch
element per core); weight matrices replicated. No collectives.

Layout: everything stays in natural (row-major) orientation.  The big
contraction is computed as T = S.T @ adj (S stationary, adj the 512-wide
moving operand), then T is PE-transposed tile-by-tile into TT so that
pooled = TT.T @ S needs no further transposes.  x is passed pre-transposed
from the host (xT) for the h-stage, whose contraction runs over F.

dtypes: the h/logits/softmax/pfeat path runs in float32r (fp32 PE mode,
2 cycles/row); the adj contraction (85% of FLOPs and HBM bytes) runs with
bf16 inputs and fp32 PSUM accumulation; since adj >= 0 and S >= 0 those
sums average the rounding noise down to ~4e-4 relative on padj.
"""

import os
from contextlib import ExitStack

import ml_dtypes
import numpy as np

import concourse.bass as bass
import concourse.tile as tile
from concourse import bacc, mybir
from concourse.masks import make_identity
from concourse.bass_utils import run_bass_kernel_spmd

B, N, F, K = 8, 2048, 512, 256
P = 128
NT = N // P   # 16 n-tiles
FT = F // P   # 4 f-tiles
KH = K // P   # 2 k-halves
NCH = 4       # n-chunks for the h-stage (512 wide)
CH = N // NCH
NCP = 19      # packed per-partition consts: b1(2) mb(16) eps(1)

F32 = mybir.dt.float32
F32R = mybir.dt.float32r
BF16 = mybir.dt.bfloat16
X = mybir.AxisListType
AF = mybir.ActivationFunctionType

# NK_ADJ_F32=1 switches the adj contraction to float32r end to end
# (fully fp32-exact, but more HBM + PE time).
ADJ_F32 = os.environ.get("NK_ADJ_F32", "0") == "1"
DT_A = F32R if ADJ_F32 else BF16
NP_A = np.float32 if ADJ_F32 else ml_dtypes.bfloat16


def _build_kernel(ctx: ExitStack, tc: tile.TileContext, io: dict):
    nc = tc.nc

    consts = ctx.enter_context(tc.tile_pool(name="consts", bufs=1))
    big = ctx.enter_context(tc.tile_pool(name="big", bufs=1))
    apool = ctx.enter_context(tc.tile_pool(name="apool", bufs=8))
    sm = ctx.enter_context(tc.tile_pool(name="sm", bufs=4))
    work = ctx.enter_context(tc.tile_pool(name="work", bufs=3))
    evp = ctx.enter_context(tc.tile_pool(name="evp", bufs=3))
    psum = ctx.enter_context(tc.tile_pool(name="psum", bufs=2, space="PSUM"))
    psum_tt = ctx.enter_context(tc.tile_pool(name="psum_tt", bufs=1, space="PSUM"))

    # ---- resident tensors / constants (DMA order = priority order) ----
    xT_sb = big.tile([P, FT, N], F32R)
    xT_r = io["xT"].rearrange("(ft p) n -> p ft n", p=P)
    nc.sync.dma_start(xT_sb[:, :, 0:CH], xT_r[:, :, 0:CH])
    w1_sb = consts.tile([P, FT, K], F32R)
    nc.sync.dma_start(w1_sb, io["w1"].rearrange("(ft p) k -> p ft k", p=P))
    cp_sb = consts.tile([P, NCP], F32)
    nc.sync.dma_start(cp_sb, io["cpack"])
    b1_sb = cp_sb[:, 0:KH]
    mb_sb = cp_sb[:, KH:KH + NT]
    eps_sb = cp_sb[:, KH + NT:KH + NT + 1]
    for c in range(1, NCH):
        nc.sync.dma_start(
            xT_sb[:, :, c * CH:(c + 1) * CH], xT_r[:, :, c * CH:(c + 1) * CH]
        )
    w2_sb = consts.tile([P, KH, K], F32R)
    nc.sync.dma_start(w2_sb, io["w2"].rearrange("(h p) k -> p h k", p=P))
    b2b_sb = consts.tile([P, K], F32)
    nc.sync.dma_start(b2b_sb, io["b2"].partition_broadcast(P))

    ident_sb = consts.tile([P, P], DT_A)
    make_identity(nc, ident_sb)
    d_sb = consts.tile([1, K], F32R)

    hT_sb = big.tile([P, KH, N], F32R)
    S_sb = big.tile([P, NT, K], F32R)
    Sa_sb = big.tile([P, NT, K], DT_A)   # S cast for the adj contraction
    T_sb = big.tile([P, KH, N], DT_A)    # S.T @ adj, k on partitions
    TT_sb = big.tile([P, NT, K], DT_A)   # its transpose, m on partitions
    x_sb = big.tile([P, NT, F], F32R)    # loaded on the 2nd HWDGE queue later
    pooled_sb = big.tile([P, KH, K], F32)

    # ---- phase 1: hT[k, n] = relu(W1.T @ xT + b1), n-chunk outer so the
    # softmax / adj pipeline can start early ----
    for nch in range(NCH):
        for kh in range(KH):
            ps = psum.tile([P, CH], F32, name="acc")
            for ft in range(FT):
                nc.tensor.matmul(
                    ps,
                    w1_sb[:, ft, kh * P:(kh + 1) * P],
                    xT_sb[:, ft, nch * CH:(nch + 1) * CH],
                    start=(ft == 0),
                    stop=(ft == FT - 1),
                )
            nc.scalar.activation(
                hT_sb[:, kh, nch * CH:(nch + 1) * CH], ps, AF.Relu,
                bias=b1_sb[:, kh:kh + 1], scale=1.0,
            )

    # ---- phase 2: logits = hT.T @ W2 + b2; S = softmax(logits + maskbias) ----
    for nt in range(NT):
        lp = psum.tile([P, K], F32, name="lg")
        for kh in range(KH):
            nc.tensor.matmul(
                lp,
                hT_sb[:, kh, nt * P:(nt + 1) * P],
                w2_sb[:, kh, :],
                start=(kh == 0),
                stop=(kh == KH - 1),
            )
        lg = work.tile([P, K], F32, name="lg_sb")
        nc.vector.tensor_add(lg, lp, b2b_sb)
        mx = sm.tile([P, 1], F32, name="mx")
        nc.vector.reduce_max(mx, lg, axis=X.X)
        eb = sm.tile([P, 1], F32, name="eb")
        nc.vector.tensor_sub(eb, mb_sb[:, nt:nt + 1], mx)  # maskbias - max
        ex = work.tile([P, K], F32, name="ex")
        ssum = sm.tile([P, 1], F32, name="ssum")
        nc.scalar.activation(ex, lg, AF.Exp, bias=eb, scale=1.0, accum_out=ssum)
        rs = sm.tile([P, 1], F32, name="rs")
        nc.vector.reciprocal(rs, ssum)
        nc.vector.tensor_scalar_mul(S_sb[:, nt, :], ex, rs)
        nc.vector.tensor_copy(Sa_sb[:, nt, :], S_sb[:, nt, :])

    # ---- phase 3: T[k, m] = S.T @ adj with S stationary and adj as the
    # wide moving operand, streamed in 1024-column half-slabs ----
    MH = 2
    MHW = N // MH
    MC = MHW // 512
    for mh in range(MH):
        tacc = {
            (kh, mc): psum_tt.tile([P, 512], F32, name=f"T{kh}{mc}")
            for kh in range(KH)
            for mc in range(MC)
        }
        for nt in range(NT):
            at = apool.tile([P, MHW], DT_A, name="aslab")
            nc.sync.dma_start(
                at, io["adj"][nt * P:(nt + 1) * P, mh * MHW:(mh + 1) * MHW]
            )
            if mh == 0 and nt % 8 == 1:
                c = nt // 8
                nc.scalar.dma_start(
                    x_sb[:, c * 8:(c + 1) * 8, :],
                    io["x"].rearrange("(t p) f -> p t f", p=P)[
                        :, c * 8:(c + 1) * 8, :
                    ],
                )
            for kh in range(KH):
                for mc in range(MC):
                    nc.tensor.matmul(
                        tacc[(kh, mc)],
                        Sa_sb[:, nt, kh * P:(kh + 1) * P],
                        at[:, mc * 512:(mc + 1) * 512],
                        start=(nt == 0),
                        stop=(nt == NT - 1),
                    )
        for kh in range(KH):
            for mc in range(MC):
                nc.vector.tensor_copy(
                    T_sb[:, kh, mh * MHW + mc * 512:mh * MHW + (mc + 1) * 512],
                    tacc[(kh, mc)],
                )

    # ---- phase 4: transpose T -> TT (PE transpose, one 128x128 block each) ----
    for mt in range(NT):
        for kh in range(KH):
            tp = psum.tile([P, K], DT_A, name="lg")
            nc.tensor.transpose(
                tp[:, :P], T_sb[:, kh, mt * P:(mt + 1) * P], ident_sb
            )
            nc.vector.tensor_copy(TT_sb[:, mt, kh * P:(kh + 1) * P], tp[:, :P])

    # ---- phase 5: pfeat = S.T @ x (fp32 exact) ----
    for kh in range(KH):
        ps = psum.tile([P, F], F32, name="acc")
        for nt in range(NT):
            nc.tensor.matmul(
                ps,
                S_sb[:, nt, kh * P:(kh + 1) * P],
                x_sb[:, nt, :],
                start=(nt == 0),
                stop=(nt == NT - 1),
            )
        pe = evp.tile([P, F], F32, name="pf_ev")
        nc.vector.tensor_copy(pe, ps)
        nc.scalar.dma_start(io["pfeat"][kh * P:(kh + 1) * P, :], pe)

    # ---- phase 6: d = 1/sqrt(row_sum + eps) as a [1, K] row vector.
    # row_sum[k] = sum_l pooled[k,l] = sum_m T[k,m] (softmax rows sum to 1),
    # reduced on DVE then transposed per 128-half on the PE. ----
    identf_sb = consts.tile([P, P], F32)
    make_identity(nc, identf_sb)
    for kh in range(KH):
        rsv = sm.tile([P, 1], F32, name="rsv")
        nc.vector.reduce_sum(rsv, T_sb[:, kh, :], axis=X.X)
        dcol = sm.tile([P, 1], F32, name="dcol")
        nc.scalar.activation(dcol, rsv, AF.Sqrt, bias=eps_sb, scale=1.0)
        nc.vector.reciprocal(dcol, dcol)
        dt = psum.tile([P, K], F32, name="lg")
        nc.tensor.transpose(dt[:1, :P], dcol.bitcast(F32), identf_sb)
        nc.vector.tensor_copy(d_sb[:1, kh * P:(kh + 1) * P], dt[:1, :P])

    # ---- phase 7: pooled = TT.T @ S; padj = pooled * (d x d) ----
    pooled_ps = []
    for kh in range(KH):
        pp = psum.tile([P, K], F32, name="lg")
        for mt in range(NT):
            nc.tensor.matmul(
                pp,
                TT_sb[:, mt, kh * P:(kh + 1) * P],
                Sa_sb[:, mt, :],
                start=(mt == 0),
                stop=(mt == NT - 1),
            )
        pooled_ps.append(pp)
    for kh in range(KH):
        nc.vector.tensor_copy(pooled_sb[:, kh, :], pooled_ps[kh])
    for kh in range(KH):
        dd = psum.tile([P, K], F32, name="lg")
        # dd[i, j] = d[kh*P+i] * d[j]  (outer product via K=1 matmul)
        nc.tensor.matmul(
            dd, d_sb[:1, kh * P:(kh + 1) * P], d_sb[:1, :], start=True, stop=True
        )
        pa = evp.tile([P, K], F32, name="pa_ev")
        nc.vector.tensor_mul(pa, pooled_sb[:, kh, :], dd)
        nc.scalar.dma_start(io["padj"][kh * P:(kh + 1) * P, :], pa)


_CACHE = {}


def _get_nc():
    if "nc" in _CACHE:
        return _CACHE["nc"]
    nc = bacc.Bacc(
        "TRN2", target_bir_lowering=False, debug=False, enable_asserts=True
    )
    io = {
        "xT": nc.dram_tensor("xT", [F, N], F32R, kind="ExternalInput").ap(),
        "x": nc.dram_tensor("x", [N, F], F32R, kind="ExternalInput").ap(),
        "adj": nc.dram_tensor("adj", [N, N], DT_A, kind="ExternalInput").ap(),
        "w1": nc.dram_tensor("w1", [F, K], F32R, kind="ExternalInput").ap(),
        "w2": nc.dram_tensor("w2", [K, K], F32R, kind="ExternalInput").ap(),
        "b2": nc.dram_tensor("b2", [K], F32, kind="ExternalInput").ap(),
        "cpack": nc.dram_tensor("cpack", [P, NCP], F32, kind="ExternalInput").ap(),
        "pfeat": nc.dram_tensor("pfeat", [K, F], F32, kind="ExternalOutput").ap(),
        "padj": nc.dram_tensor("padj", [K, K], F32, kind="ExternalOutput").ap(),
    }
    with tile.TileContext(nc) as tc, ExitStack() as ctx:
        _build_kernel(ctx, tc, io)
    nc.compile()
    _CACHE["nc"] = nc
    return nc


def make_in_maps(x, adj, mask, W1, b1, W2, b2):
    """Build the per-core input maps from the full (unsharded) inputs."""
    x = np.asarray(x, np.float32)
    adj = np.asarray(adj, np.float32)
    mask = np.asarray(mask, np.float32)
    w1 = np.ascontiguousarray(np.asarray(W1, np.float32))
    w2 = np.ascontiguousarray(np.asarray(W2, np.float32))
    b1v = np.asarray(b1, np.float32).reshape(K)
    b2v = np.ascontiguousarray(np.asarray(b2, np.float32).reshape(K))
    in_maps = []
    for b in range(B):
        xb = np.ascontiguousarray(x[b])
        mb = (-1e9 * (1.0 - mask[b])).astype(np.float32)
        cpack = np.empty((P, NCP), np.float32)
        cpack[:, 0] = b1v[0:P]
        cpack[:, 1] = b1v[P:2 * P]
        for t in range(NT):
            cpack[:, KH + t] = mb[t * P:(t + 1) * P]
        cpack[:, KH + NT] = 1e-9
        in_maps.append({
            "xT": np.ascontiguousarray(xb.T),
            "x": xb,
            "adj": np.ascontiguousarray(adj[b].astype(NP_A)),
            "w1": w1,
            "w2": w2,
            "b2": b2v,
            "cpack": cpack,
        })
    return in_maps


def run(x, adj, mask, W1, b1, W2, b2, trace=False):
    nc = _get_nc()
    in_maps = make_in_maps(x, adj, mask, W1, b1, W2, b2)
    res = run_bass_kernel_spmd(nc, in_maps, core_ids=list(range(B)), trace=trace)
    pfeat = np.stack([res.results[b]["pfeat"] for b in range(B)]).astype(np.float32)
    padj = np.stack([res.results[b]["padj"] for b in range(B)]).astype(np.float32)
    pmask = np.ones((B, K), np.float32)
    return (pfeat, padj, pmask), res


def kernel(x, adj, mask, W1, b1, W2, b2):
    out, _ = run(x, adj, mask, W1, b1, W2, b2, trace=False)
    return out


# revision 11
# speedup vs baseline: 1.2855x; 1.1711x over previous
"""Trainium2 Bass kernel for nn_NodeAggregator (gnn message passing / diffpool-style).

Reference math (per batch element b, forward pass only):
    h      = relu(x @ W1 + b1)                      [N, K]
    logits = h @ W2 + b2 + (-1e9)*(1-mask)[:,None]  [N, K]
    S      = softmax(logits, axis=-1)               [N, K]
    pfeat  = S.T @ x                                [K, F]
    pooled = S.T @ adj @ S                          [K, K]
    (threshold/topk/scatter + straight-through estimator is an exact
     no-op in the forward pass: a_sp + (pooled - a_sp) == pooled)
    d      = 1/sqrt(pooled.sum(-1) + 1e-9)
    padj   = pooled * d[:,None] * d[None,:]
    pmask  = ones

Sharding: data-parallel over batch B=8 across the 8 NeuronCores (one batch
element per core); weights replicated; no collectives.

Layout: everything stays in natural (row-major) orientation.  The big
contraction is computed as T = S.T @ adj (S stationary, adj the 512-wide
moving operand), then T is PE-transposed tile-by-tile into TT so that
pooled = TT.T @ S needs no further data movement.  x is passed
pre-transposed from the host (xT) for the h-stage, whose contraction runs
over F.  All inputs are host-retiled so that every DMA is a fully
contiguous per-partition transfer.

dtypes: matmul inputs in bf16 with fp32 PSUM accumulation; softmax and the
degree renormalization in fp32.  Measured ~2.6e-3 max relative error vs
the fp32 reference.  NK_F32=1 switches the MLP/softmax/pfeat path to
float32r (fp32-exact PE mode, ~4e-4 total error, ~25% slower).
"""

import os
from contextlib import ExitStack

import ml_dtypes
import numpy as np

import concourse.bass as bass
import concourse.tile as tile
from concourse import bacc, mybir
from concourse.masks import make_identity
from concourse.bass_utils import run_bass_kernel_spmd

B, N, F, K = 8, 2048, 512, 256
P = 128
NT = N // P   # 16 n-tiles
FT = F // P   # 4 f-tiles
KH = K // P   # 2 k-halves
NCH = 4       # xT n-chunks for the h-stage (512 wide)
CH = N // NCH
XC = 2        # x chunks
MH = 2        # adj column halves
MHW = N // MH
MC = MHW // 512
NCP = 19      # packed per-partition consts: b1(2) mb(16) eps(1)

F32 = mybir.dt.float32
F32R = mybir.dt.float32r
BF16 = mybir.dt.bfloat16
X = mybir.AxisListType
AF = mybir.ActivationFunctionType

F32_MODE = os.environ.get("NK_F32", "0") == "1"
DT_M = F32R if F32_MODE else BF16            # MLP/pfeat matmul dtype
NP_M = np.float32 if F32_MODE else ml_dtypes.bfloat16
DT_A = BF16                                   # adj-contraction dtype
NP_A = ml_dtypes.bfloat16


def _build_kernel(ctx: ExitStack, tc: tile.TileContext, io: dict):
    nc = tc.nc

    consts = ctx.enter_context(tc.tile_pool(name="consts", bufs=1))
    big = ctx.enter_context(tc.tile_pool(name="big", bufs=1))
    apool = ctx.enter_context(tc.tile_pool(name="apool", bufs=8))
    sm = ctx.enter_context(tc.tile_pool(name="sm", bufs=4))
    work = ctx.enter_context(tc.tile_pool(name="work", bufs=3))
    evp = ctx.enter_context(tc.tile_pool(name="evp", bufs=3))
    psum = ctx.enter_context(tc.tile_pool(name="psum", bufs=2, space="PSUM"))
    psum_tt = ctx.enter_context(tc.tile_pool(name="psum_tt", bufs=1, space="PSUM"))

    # ---- resident tensors / constants (DMA order = priority order) ----
    xT_sb = big.tile([P, FT, N], DT_M)
    nc.sync.dma_start(xT_sb[:, :, 0:CH], io["xT"][0])
    w1_sb = consts.tile([P, FT, K], DT_M)
    nc.sync.dma_start(w1_sb, io["w1"])
    cp_sb = consts.tile([P, NCP], F32)
    nc.sync.dma_start(cp_sb, io["cpack"])
    b1_sb = cp_sb[:, 0:KH]
    mb_sb = cp_sb[:, KH:KH + NT]
    eps_sb = cp_sb[:, KH + NT:KH + NT + 1]
    for c in range(1, NCH):
        nc.sync.dma_start(xT_sb[:, :, c * CH:(c + 1) * CH], io["xT"][c])
    w2_sb = consts.tile([P, KH, K], DT_M)
    nc.sync.dma_start(w2_sb, io["w2"])
    b2b_sb = consts.tile([P, K], F32)
    nc.sync.dma_start(b2b_sb, io["b2"].partition_broadcast(P))

    ident_sb = consts.tile([P, P], DT_A)
    make_identity(nc, ident_sb)
    d_sb = consts.tile([1, K], F32R)

    hT_sb = big.tile([P, KH, N], DT_M)
    S_sb = big.tile([P, NT, K], DT_M) if F32_MODE else None
    Sa_sb = big.tile([P, NT, K], DT_A)   # S in the adj-contraction dtype
    T_sb = big.tile([P, KH, N], DT_A)    # S.T @ adj, k on partitions
    TT_sb = big.tile([P, NT, K], DT_A)   # its transpose, m on partitions
    x_sb = big.tile([P, NT, F], DT_M)    # loaded on the 2nd HWDGE queue later
    pooled_sb = big.tile([P, KH, K], F32)
    Sp_sb = S_sb if F32_MODE else Sa_sb  # pfeat stationary operand

    # ---- phase 1: hT[k, n] = relu(W1.T @ xT + b1), n-chunk outer so the
    # softmax / adj pipeline can start early ----
    for nch in range(NCH):
        for kh in range(KH):
            ps = psum.tile([P, CH], F32, name="acc")
            for ft in range(FT):
                nc.tensor.matmul(
                    ps,
                    w1_sb[:, ft, kh * P:(kh + 1) * P],
                    xT_sb[:, ft, nch * CH:(nch + 1) * CH],
                    start=(ft == 0),
                    stop=(ft == FT - 1),
                )
            nc.scalar.activation(
                hT_sb[:, kh, nch * CH:(nch + 1) * CH], ps, AF.Relu,
                bias=b1_sb[:, kh:kh + 1], scale=1.0,
            )

    # ---- phase 2: logits = hT.T @ W2 + b2; S = softmax(logits + maskbias) ----
    for nt in range(NT):
        lp = psum.tile([P, K], F32, name="lg")
        for kh in range(KH):
            nc.tensor.matmul(
                lp,
                hT_sb[:, kh, nt * P:(nt + 1) * P],
                w2_sb[:, kh, :],
                start=(kh == 0),
                stop=(kh == KH - 1),
            )
        lg = work.tile([P, K], F32, name="lg_sb")
        nc.vector.tensor_add(lg, lp, b2b_sb)
        mx = sm.tile([P, 1], F32, name="mx")
        nc.vector.reduce_max(mx, lg, axis=X.X)
        eb = sm.tile([P, 1], F32, name="eb")
        nc.vector.tensor_sub(eb, mb_sb[:, nt:nt + 1], mx)  # maskbias - max
        ex = work.tile([P, K], F32, name="ex")
        ssum = sm.tile([P, 1], F32, name="ssum")
        nc.scalar.activation(ex, lg, AF.Exp, bias=eb, scale=1.0, accum_out=ssum)
        rs = sm.tile([P, 1], F32, name="rs")
        nc.vector.reciprocal(rs, ssum)
        if F32_MODE:
            nc.vector.tensor_scalar_mul(S_sb[:, nt, :], ex, rs)
            nc.vector.tensor_copy(Sa_sb[:, nt, :], S_sb[:, nt, :])
        else:
            nc.vector.tensor_scalar_mul(Sa_sb[:, nt, :], ex, rs)

    # ---- phase 3: T[k, m] = S.T @ adj with S stationary and adj as the
    # wide moving operand, streamed in 1024-column half-slabs ----
    for mh in range(MH):
        tacc = {
            (kh, mc): psum_tt.tile([P, 512], F32, name=f"T{kh}{mc}")
            for kh in range(KH)
            for mc in range(MC)
        }
        for nt in range(NT):
            at = apool.tile([P, MHW], DT_A, name="aslab")
            nc.sync.dma_start(at, io["adj"][mh, nt])
            if mh == 0 and nt % 8 == 1:
                c = nt // 8
                nc.scalar.dma_start(x_sb[:, c * 8:(c + 1) * 8, :], io["x"][c])
            for kh in range(KH):
                for mc in range(MC):
                    nc.tensor.matmul(
                        tacc[(kh, mc)],
                        Sa_sb[:, nt, kh * P:(kh + 1) * P],
                        at[:, mc * 512:(mc + 1) * 512],
                        start=(nt == 0),
                        stop=(nt == NT - 1),
                    )
        for kh in range(KH):
            for mc in range(MC):
                nc.vector.tensor_copy(
                    T_sb[:, kh, mh * MHW + mc * 512:mh * MHW + (mc + 1) * 512],
                    tacc[(kh, mc)],
                )

    # ---- phase 4: transpose T -> TT (PE transpose, one 128x128 block each) ----
    for mt in range(NT):
        for kh in range(KH):
            tp = psum.tile([P, K], DT_A, name="lg")
            nc.tensor.transpose(
                tp[:, :P], T_sb[:, kh, mt * P:(mt + 1) * P], ident_sb
            )
            nc.vector.tensor_copy(TT_sb[:, mt, kh * P:(kh + 1) * P], tp[:, :P])

    # ---- phase 5: pfeat = S.T @ x ----
    for kh in range(KH):
        ps = psum.tile([P, F], F32, name="acc")
        for nt in range(NT):
            nc.tensor.matmul(
                ps,
                Sp_sb[:, nt, kh * P:(kh + 1) * P],
                x_sb[:, nt, :],
                start=(nt == 0),
                stop=(nt == NT - 1),
            )
        pe = evp.tile([P, F], F32, name="pf_ev")
        nc.vector.tensor_copy(pe, ps)
        nc.scalar.dma_start(io["pfeat"][kh * P:(kh + 1) * P, :], pe)

    # ---- phase 6: d = 1/sqrt(row_sum + eps) as a [1, K] row vector.
    # row_sum[k] = sum_l pooled[k,l] = sum_m T[k,m] (softmax rows sum to 1),
    # reduced on DVE then transposed per 128-half on the PE. ----
    identf_sb = consts.tile([P, P], F32)
    make_identity(nc, identf_sb)
    for kh in range(KH):
        rsv = sm.tile([P, 1], F32, name="rsv")
        nc.vector.reduce_sum(rsv, T_sb[:, kh, :], axis=X.X)
        dcol = sm.tile([P, 1], F32, name="dcol")
        nc.scalar.activation(dcol, rsv, AF.Sqrt, bias=eps_sb, scale=1.0)
        nc.vector.reciprocal(dcol, dcol)
        dt = psum.tile([P, K], F32, name="lg")
        nc.tensor.transpose(dt[:1, :P], dcol, identf_sb)
        nc.vector.tensor_copy(d_sb[:1, kh * P:(kh + 1) * P], dt[:1, :P])

    # ---- phase 7: pooled = TT.T @ S; padj = pooled * (d x d) ----
    pooled_ps = []
    for kh in range(KH):
        pp = psum.tile([P, K], F32, name="lg")
        for mt in range(NT):
            nc.tensor.matmul(
                pp,
                TT_sb[:, mt, kh * P:(kh + 1) * P],
                Sa_sb[:, mt, :],
                start=(mt == 0),
                stop=(mt == NT - 1),
            )
        pooled_ps.append(pp)
    for kh in range(KH):
        nc.vector.tensor_copy(pooled_sb[:, kh, :], pooled_ps[kh])
    for kh in range(KH):
        dd = psum.tile([P, K], F32, name="lg")
        # dd[i, j] = d[kh*P+i] * d[j]  (outer product via K=1 matmul)
        nc.tensor.matmul(
            dd, d_sb[:1, kh * P:(kh + 1) * P], d_sb[:1, :], start=True, stop=True
        )
        pa = evp.tile([P, K], F32, name="pa_ev")
        nc.vector.tensor_mul(pa, pooled_sb[:, kh, :], dd)
        nc.scalar.dma_start(io["padj"][kh * P:(kh + 1) * P, :], pa)


_CACHE = {}


def _get_nc():
    if "nc" in _CACHE:
        return _CACHE["nc"]
    nc = bacc.Bacc(
        "TRN2", target_bir_lowering=False, debug=False, enable_asserts=True
    )
    io = {
        "xT": nc.dram_tensor("xT", [NCH, P, FT, CH], DT_M, kind="ExternalInput").ap(),
        "x": nc.dram_tensor("x", [XC, P, NT // XC, F], DT_M, kind="ExternalInput").ap(),
        "adj": nc.dram_tensor("adj", [MH, NT, P, MHW], DT_A, kind="ExternalInput").ap(),
        "w1": nc.dram_tensor("w1", [P, FT, K], DT_M, kind="ExternalInput").ap(),
        "w2": nc.dram_tensor("w2", [P, KH, K], DT_M, kind="ExternalInput").ap(),
        "b2": nc.dram_tensor("b2", [K], F32, kind="ExternalInput").ap(),
        "cpack": nc.dram_tensor("cpack", [P, NCP], F32, kind="ExternalInput").ap(),
        "pfeat": nc.dram_tensor("pfeat", [K, F], F32, kind="ExternalOutput").ap(),
        "padj": nc.dram_tensor("padj", [K, K], F32, kind="ExternalOutput").ap(),
    }
    with tile.TileContext(nc) as tc, ExitStack() as ctx:
        _build_kernel(ctx, tc, io)
    nc.compile()
    _CACHE["nc"] = nc
    return nc


def make_in_maps(x, adj, mask, W1, b1, W2, b2):
    """Build the per-core input maps from the full (unsharded) inputs.

    All matmul operands are host-retiled into [chunk][partition][...]
    layouts so every device DMA is a fully contiguous per-partition read.
    """
    x = np.asarray(x, np.float32)
    adj = np.asarray(adj, np.float32)
    mask = np.asarray(mask, np.float32)
    w1 = np.asarray(W1, np.float32).astype(NP_M)
    w2 = np.asarray(W2, np.float32).astype(NP_M)
    b1v = np.asarray(b1, np.float32).reshape(K)
    b2v = np.ascontiguousarray(np.asarray(b2, np.float32).reshape(K))
    w1_t = np.ascontiguousarray(w1.reshape(FT, P, K).transpose(1, 0, 2))
    w2_t = np.ascontiguousarray(w2.reshape(KH, P, K).transpose(1, 0, 2))
    in_maps = []
    for b in range(B):
        xb = x[b]
        xm = xb.astype(NP_M)
        xT_t = np.ascontiguousarray(
            xm.T.reshape(FT, P, NCH, CH).transpose(2, 1, 0, 3)
        )
        x_t = np.ascontiguousarray(
            xm.reshape(XC, NT // XC, P, F).transpose(0, 2, 1, 3)
        )
        adj_t = np.ascontiguousarray(
            adj[b].astype(NP_A).reshape(NT, P, MH, MHW).transpose(2, 0, 1, 3)
        )
        mb = (-1e9 * (1.0 - mask[b])).astype(np.float32)
        cpack = np.empty((P, NCP), np.float32)
        cpack[:, 0] = b1v[0:P]
        cpack[:, 1] = b1v[P:2 * P]
        for t in range(NT):
            cpack[:, KH + t] = mb[t * P:(t + 1) * P]
        cpack[:, KH + NT] = 1e-9
        in_maps.append({
            "xT": xT_t,
            "x": x_t,
            "adj": adj_t,
            "w1": w1_t,
            "w2": w2_t,
            "b2": b2v,
            "cpack": cpack,
        })
    return in_maps


def run(x, adj, mask, W1, b1, W2, b2, trace=False):
    nc = _get_nc()
    in_maps = make_in_maps(x, adj, mask, W1, b1, W2, b2)
    res = run_bass_kernel_spmd(nc, in_maps, core_ids=list(range(B)), trace=trace)
    pfeat = np.stack([res.results[b]["pfeat"] for b in range(B)]).astype(np.float32)
    padj = np.stack([res.results[b]["padj"] for b in range(B)]).astype(np.float32)
    pmask = np.ones((B, K), np.float32)
    return (pfeat, padj, pmask), res


def kernel(x, adj, mask, W1, b1, W2, b2):
    out, _ = run(x, adj, mask, W1, b1, W2, b2, trace=False)
    return out
